# revision 21
# baseline (speedup 1.0000x reference)
"""Trainium2 Bass kernel for nn_CellLineGraphEncoder (GENConv x3 + pooling).

Strategy (8 NeuronCores, SPMD):
  - Nodes are sharded: core k owns 49 blocks of 128 nodes (6272 nodes).
  - Edges are sharded by dst node owner; per (core, dst-block) the edges are
    split into lo/hi halves by gather-table row (int16 index limit) and each
    half is padded to a uniform chunk count (K_LO / K_HI chunks of 128) so a
    single program fits all cores.
  - Per layer: all-gather a bf16 z-table (conv input features + b_edge,
    rows padded to 256B for the gather), bulk-gather z[src] rows per edge via
    dma_gather, compute softmax-aggregation messages on ACT/DVE, segment-
    reduce per dst-block with one-hot matmuls on PE (S1 = sum exp,
    S2 = sum exp*msg), then agg = S2/(S1+eps), residual, and the
    64->128->64 MLP with feature-dim LayerNorm.
  - Softmax max-subtraction is dropped (messages are bounded ~7, exp is safe;
    softmax is shift-invariant so the result matches the reference).
  - Final avg/max pooling is reduced per core on device; the cross-core
    combine and the last [1,64] @ [64,64] linear run on host.
"""

import math
import numpy as np

import concourse.bass as bass
import concourse.bacc as bacc
import concourse.mybir as mybir
import concourse.tile as tile
from concourse.bass_utils import run_bass_kernel_spmd
from concourse.masks import make_identity

P = 128
NCORES = 8
N_NODES = 50000
N_EDGES = 1000000
HID = 64
L = 3
NBLK = 49                   # dst-node blocks per core
NPC = NBLK * P              # nodes per core (6272)
NTOT = NCORES * NPC         # padded node count (50176)
ROWW = 128                  # padded table row width (bf16 -> 256B)
HI_BASE = 32768             # int16 index split point
EPS_MSG = 1e-7
EPS_SM = 1e-16
LN_EPS = 1e-5
GB = 1                      # dst-blocks per gather group

F32 = mybir.dt.float32
BF16 = mybir.dt.bfloat16
I16 = mybir.dt.int16
I32 = mybir.dt.int32
OP = mybir.AluOpType
ACT = mybir.ActivationFunctionType

# 6272 = 12*512 + 128
SLICES = [(i * 512, 512) for i in range(12)] + [(6144, 128)]
GROUPS = [list(range(g, min(g + GB, NBLK))) for g in range(0, NBLK, GB)]


def _table_row(node):
    """Row of a node in the all-gathered z table (p-major per-core layout)."""
    kc = node // NPC
    loc = node - kc * NPC
    p = loc % P
    b = loc // P
    return kc * NPC + p * NBLK + b


def _prep_edges(edge_index, edge_attr):
    """Pair-packed table: 2 nodes per 256B row -> 25088 rows, unsigned int16
    idx covers the whole table. Per (core, block) edges are split by src-row
    parity (even rows first, then odd) so each chunk reads the low or high
    64 features of its gathered pair row via a pure AP offset."""
    src = edge_index[0].astype(np.int64)
    dst = edge_index[1].astype(np.int64)
    core = dst // NPC
    blk = (dst % NPC) // P
    dl = ((dst % NPC) % P).astype(np.float32)
    row = _table_row(src)
    pr = row >> 1               # pair row, 0..25087
    par = (row & 1).astype(np.int64)
    g = core * NBLK + blk
    av = edge_attr[:, 0].astype(np.float32)

    gh = g * 2 + par
    counts = np.bincount(gh, minlength=NCORES * NBLK * 2)
    cnt2 = counts.reshape(NCORES, NBLK, 2)
    cnt_e = cnt2[:, :, 0].max(axis=0).astype(int)
    cnt_o = cnt2[:, :, 1].max(axis=0).astype(int)
    K_E = [max(1, int(math.ceil(c / P))) for c in cnt_e]
    K_O = [max(1, int(math.ceil(c / P))) for c in cnt_o]
    choff = np.zeros(NBLK + 1, np.int64)
    for b in range(NBLK):
        choff[b + 1] = choff[b] + K_E[b] + K_O[b]
    nchk = int(choff[NBLK])
    # valid gather span per block (trailing strip hits only -1 slots)
    reg = [K_E[b] * P + int(cnt_o[b]) for b in range(NBLK)]

    order = np.argsort(gh, kind="stable")
    starts = np.zeros(NCORES * NBLK * 2 + 1, np.int64)
    np.cumsum(counts, out=starts[1:])
    pr_s, dl_s, av_s = pr[order], dl[order], av[order]

    nslot = nchk * P
    idx_c, dl_c, av_c = [], [], []
    for c in range(NCORES):
        idxa = np.full(nslot, -1, np.int32)
        dla = np.full(nslot, -1.0, np.float32)
        ava = np.zeros(nslot, np.float32)
        for b in range(NBLK):
            for half, K_half, cmax_a in ((0, K_E, cnt_e), (1, K_O, cnt_o)):
                gi = (c * NBLK + b) * 2 + half
                s, e = starts[gi], starts[gi + 1]
                n = e - s
                o = (choff[b] + (K_E[b] if half else 0)) * P
                cmax = cmax_a[b]
                idxa[o:o + cmax] = 0
                idxa[o:o + n] = pr_s[s:e]
                dla[o:o + n] = dl_s[s:e]
                ava[o:o + n] = av_s[s:e]
        # value arrays: slot j -> (lane j%128, chunk j//128)
        dl_c.append(np.ascontiguousarray(dla.reshape(nchk, P).T))
        av_c.append(np.ascontiguousarray(ava.reshape(nchk, P).T))
        # index array: slot j -> (partition j%16, free j//16), replicated x8
        i16 = idxa.astype(np.int16).reshape(-1, 16).T   # [16, nslot/16]
        idx_c.append(np.ascontiguousarray(np.tile(i16, (8, 1))))
    return K_E, K_O, reg, idx_c, dl_c, av_c


def _build(K_E, K_O, reg, dbg=False):
    choff = [0]
    for b in range(NBLK):
        choff.append(choff[b] + K_E[b] + K_O[b])
    nchk = choff[NBLK]
    nslot = nchk * P
    KEMAX = max(K_E)
    KOMAX = max(K_O)
    KMAX = max(K_E[b] + K_O[b] for b in range(NBLK))
    nc = bacc.Bacc("TRN2", target_bir_lowering=False, debug=False,
                   num_devices=NCORES, num_swdge_queues=4)
    if dbg:
        d_dbg_h = nc.dram_tensor("dbg_h", [HID, NPC], F32,
                                 kind="ExternalOutput").ap()
        d_dbg_tab = nc.dram_tensor("dbg_tab", [NTOT, ROWW], BF16,
                                   kind="ExternalOutput").ap()
        d_dbg_g = nc.dram_tensor("dbg_g", [P, 2, ROWW], BF16,
                                 kind="ExternalOutput").ap()
        d_dbg_s = nc.dram_tensor("dbg_s", [P, 2 * HID], F32,
                                 kind="ExternalOutput").ap()

    d_idx = nc.dram_tensor("eidx16", [P, nslot // 16], I16,
                           kind="ExternalInput").ap()
    d_edl = nc.dram_tensor("edl", [P, nchk], F32, kind="ExternalInput").ap()
    d_eav = nc.dram_tensor("eav", [P, nchk], F32, kind="ExternalInput").ap()
    d_xp = nc.dram_tensor("xp", [NPC, 16], F32, kind="ExternalInput").ap()
    d_wnode = nc.dram_tensor("wnode", [16, HID], F32, kind="ExternalInput").ap()
    d_bnode = nc.dram_tensor("bnode", [HID, 1], F32, kind="ExternalInput").ap()
    d_web = nc.dram_tensor("web", [P, HID], F32, kind="ExternalInput").ap()
    d_benm = nc.dram_tensor("benm", [P, HID], F32, kind="ExternalInput").ap()
    d_w1 = nc.dram_tensor("w1", [L, HID, 2 * HID], F32, kind="ExternalInput").ap()
    d_w2 = nc.dram_tensor("w2", [L, 2 * HID, HID], F32, kind="ExternalInput").ap()
    d_b1c = nc.dram_tensor("b1c", [2 * HID, L], F32, kind="ExternalInput").ap()
    d_g1c = nc.dram_tensor("g1c", [2 * HID, L], F32, kind="ExternalInput").ap()
    d_be1c = nc.dram_tensor("be1c", [2 * HID, L], F32, kind="ExternalInput").ap()
    d_b2c = nc.dram_tensor("b2c", [HID, L], F32, kind="ExternalInput").ap()
    d_ngc = nc.dram_tensor("ngc", [HID, L], F32, kind="ExternalInput").ap()
    d_nbc = nc.dram_tensor("nbc", [HID, L], F32, kind="ExternalInput").ap()
    d_tc = nc.dram_tensor("tcol", [P, L], F32, kind="ExternalInput").ap()
    d_out = nc.dram_tensor("pool_out", [HID, 2], F32, kind="ExternalOutput").ap()

    with tile.TileContext(nc) as tc:
        with (
            tc.tile_pool(name="cpool", bufs=1) as cpool,
            tc.tile_pool(name="gpool", bufs=2) as gpool,
            tc.tile_pool(name="epool", bufs=3) as epool,
            tc.tile_pool(name="vpool", bufs=2) as vpool,
            tc.tile_pool(name="opool", bufs=6) as opool,
            tc.tile_pool(name="npool", bufs=3) as npool,
            tc.tile_pool(name="mpool", bufs=2) as mpool,
            tc.tile_pool(name="spool", bufs=2) as spool,
            tc.tile_pool(name="pmm", bufs=2, space="PSUM") as pmm,
            tc.tile_pool(name="ptp", bufs=2, space="PSUM") as ptp,
            tc.tile_pool(name="paux", bufs=4, space="PSUM") as paux,
            tc.tile_pool(name="dpool", bufs=1, space="DRAM") as dpool,
        ):
            # ---- constants / persistent state ----
            iota_i = cpool.tile([P, P], I32)
            nc.gpsimd.iota(iota_i[:], pattern=[[1, P]], base=0,
                           channel_multiplier=0)
            iota_f = cpool.tile([P, P], F32)
            nc.vector.tensor_copy(iota_f[:], iota_i[:])
            ident = cpool.tile([P, P], F32)
            make_identity(nc, ident[:])
            ones_col = cpool.tile([P, 1], F32)
            nc.vector.memset(ones_col[:], 1.0)
            ones_row = cpool.tile([1, P], F32)
            nc.vector.memset(ones_row[:], 1.0)
            ident_hi = cpool.tile([P, HID], F32)
            nc.vector.memset(ident_hi[:], 0.0)
            nc.sync.dma_start(out=ident_hi[HID:P, :], in_=ident[:HID, :HID])
            epsm_col = cpool.tile([P, 1], F32)
            nc.vector.memset(epsm_col[:], EPS_MSG)
            lneps_col = cpool.tile([P, 1], F32)
            nc.vector.memset(lneps_col[:], LN_EPS)

            def load(shape, dtype, src_ap, name):
                t = cpool.tile(shape, dtype, name=name)
                nc.sync.dma_start(out=t[:], in_=src_ap)
                return t

            idx16 = load([P, nslot // 16], I16, d_idx, "idx16_s")
            edl = load([P, nchk], F32, d_edl, "edl_s")
            eav = load([P, nchk], F32, d_eav, "eav_s")
            wnode = load([16, HID], F32, d_wnode, "wnode_s")
            bnode = load([HID, 1], F32, d_bnode, "bnode_s")
            web = load([P, HID], F32, d_web, "web_s")
            benm = load([P, HID], F32, d_benm, "benm_s")
            w1 = [load([HID, 2 * HID], F32, d_w1[l], f"w1_{l}") for l in range(L)]
            w2 = [load([2 * HID, HID], F32, d_w2[l], f"w2_{l}") for l in range(L)]
            b1c = load([2 * HID, L], F32, d_b1c, "b1c_s")
            g1c = load([2 * HID, L], F32, d_g1c, "g1c_s")
            be1c = load([2 * HID, L], F32, d_be1c, "be1c_s")
            b2c = load([HID, L], F32, d_b2c, "b2c_s")
            ngc = load([HID, L], F32, d_ngc, "ngc_s")
            nbc = load([HID, L], F32, d_nbc, "nbc_s")
            tcol = load([P, L], F32, d_tc, "tcol_s")

            h = cpool.tile([HID, NPC], F32)          # feature-major node state
            zrows = cpool.tile([P, NBLK * HID], F32)  # z node-major rows
            zbb = cpool.tile([P, NBLK, HID], BF16)    # z + b_edge rows (packed)
            nc.vector.memset(zbb[:], 0.0)
            gts = [cpool.tile([P, KMAX, 2 * HID], BF16, name=f"gt{i}")
                   for i in range(3)]
            for g_ in gts:
                nc.vector.memset(g_[:], 0.0)
            # in1T rows 0:64 = MLP input (fmajor); rows 64:128 = zf scratch
            in1T = cpool.tile([P, NPC], F32)
            zf = in1T[64:128, :]

            zbounce = [dpool.tile([NPC, HID], BF16, name=f"zbounce{l}")
                       for l in range(L)]
            ztable = [dpool.tile([NTOT, HID], BF16, name=f"ztable{l}",
                                 addr_space="Shared") for l in range(L)]

            # ---- h0 = x @ W_node + b_node (feature-major) ----
            xr, xr_free = tc.tile([P, NBLK * 16], F32, name="xr")
            nc.sync.dma_start(
                out=xr[:],
                in_=d_xp.rearrange("(p b) f -> p (b f)", p=P))
            for b in range(NBLK):
                pt = ptp.tile([16, P], F32, name="pt16", tag="tp")
                nc.tensor.transpose(pt[:], xr[:, b * 16:(b + 1) * 16], ident[:])
                nc.vector.tensor_copy(in1T[:16, b * P:(b + 1) * P], pt[:])
            for c0, w in SLICES:
                pmlp = pmm.tile([HID, 512], F32, name="p_h0", tag="mm")
                nc.tensor.matmul(pmlp[:, :w], lhsT=wnode[:],
                                 rhs=in1T[:16, c0:c0 + w], start=True, stop=True)
                nc.vector.tensor_scalar(out=h[:, c0:c0 + w], in0=pmlp[:, :w],
                                        scalar1=bnode[:, 0:1], scalar2=None,
                                        op0=OP.add)
            xr_free()
            if dbg:
                nc.sync.dma_start(out=d_dbg_h, in_=h[:])

            # ---- helper: feature-major LayerNorm (+ relu) over partition dim
            def ln_partition(dst, src_t, nfeat, g_col, b_col):
                inv = 1.0 / nfeat
                for c0, w in SLICES:
                    sq = spool.tile([nfeat, 512], F32, name="lnsq", tag="lnsq")
                    nc.scalar.activation(sq[:, :w], src_t[:nfeat, c0:c0 + w],
                                         ACT.Square)
                    pm = paux.tile([1, 512], F32, name="pm", tag="aux")
                    pq = paux.tile([1, 512], F32, name="pq", tag="aux")
                    nc.tensor.matmul(pm[:, :w], lhsT=ones_col[:nfeat, :],
                                     rhs=src_t[:nfeat, c0:c0 + w],
                                     start=True, stop=True)
                    nc.tensor.matmul(pq[:, :w], lhsT=ones_col[:nfeat, :],
                                     rhs=sq[:, :w], start=True, stop=True)
                    stA = spool.tile([1, 512], F32, name="stA", tag="stA")
                    stB = spool.tile([1, 512], F32, name="stB", tag="stB")
                    tmp = spool.tile([1, 512], F32, name="sttmp", tag="sttmp")
                    nc.vector.tensor_scalar(out=stA[:, :w], in0=pq[:, :w],
                                            scalar1=inv, scalar2=None, op0=OP.mult)
                    nc.vector.tensor_scalar(out=stB[:, :w], in0=pm[:, :w],
                                            scalar1=inv, scalar2=None, op0=OP.mult)
                    nc.vector.tensor_tensor(out=tmp[:, :w], in0=stB[:, :w],
                                            in1=stB[:, :w], op=OP.mult)
                    nc.vector.tensor_tensor(out=stA[:, :w], in0=stA[:, :w],
                                            in1=tmp[:, :w], op=OP.subtract)
                    nc.scalar.activation(stA[:, :w], stA[:, :w], ACT.Sqrt,
                                         bias=lneps_col[:1, :])
                    nc.vector.reciprocal(stA[:, :w], stA[:, :w])   # rstd
                    nc.vector.tensor_tensor(out=stB[:, :w], in0=stB[:, :w],
                                            in1=stA[:, :w], op=OP.mult)
                    pA = paux.tile([nfeat, 512], F32, name="pA", tag="aux")
                    pB = paux.tile([nfeat, 512], F32, name="pB", tag="aux")
                    nc.tensor.matmul(pA[:, :w], lhsT=ones_row[:, :nfeat],
                                     rhs=stA[:, :w], start=True, stop=True)
                    nc.tensor.matmul(pB[:, :w], lhsT=ones_row[:, :nfeat],
                                     rhs=stB[:, :w], start=True, stop=True)
                    u = spool.tile([nfeat, 512], F32, name="lnu", tag="lnu")
                    nc.vector.tensor_tensor(out=u[:, :w],
                                            in0=src_t[:nfeat, c0:c0 + w],
                                            in1=pA[:, :w], op=OP.mult)
                    nc.vector.tensor_tensor(out=u[:, :w], in0=u[:, :w],
                                            in1=pB[:, :w], op=OP.subtract)
                    nc.vector.tensor_scalar(out=u[:, :w], in0=u[:, :w],
                                            scalar1=g_col, scalar2=b_col,
                                            op0=OP.mult, op1=OP.add)
                    nc.scalar.activation(dst[:nfeat, c0:c0 + w], u[:, :w],
                                         ACT.Relu)

            # ---- per-chunk-range edge elementwise: returns val tile ----
            def edge_values(gt, gofs, c0, ncnk, kmax, fofs, tag):
                """gt[:, gofs:gofs+ncnk, fofs:fofs+64] + chunks [c0, c0+ncnk)."""
                ea_f = epool.tile([P, kmax, HID], BF16, name=f"ea{tag}",
                                  tag=f"ea{tag}")
                ea = ea_f[:, 0:ncnk, :]
                nc.vector.tensor_tensor(
                    out=ea,
                    in0=web[:].rearrange("p (o f) -> p o f", o=1)
                        .to_broadcast([P, ncnk, HID]),
                    in1=eav[:, c0:c0 + ncnk]
                        .rearrange("p (k o) -> p k o", o=1)
                        .to_broadcast([P, ncnk, HID]),
                    op=OP.mult)
                nc.vector.tensor_tensor(
                    out=ea, in0=ea,
                    in1=gt[:, gofs:gofs + ncnk, fofs:fofs + HID], op=OP.add)
                nc.scalar.activation(ea, ea, ACT.Relu,
                                     bias=epsm_col[:])   # msg
                val_f = vpool.tile([P, kmax, 2, HID], BF16, name=f"val{tag}",
                                   tag=f"val{tag}")
                val = val_f[:, 0:ncnk, :, :]
                nc.scalar.activation(val[:, :, 0, :], ea, ACT.Exp,
                                     scale=tcol[:, lay:lay + 1])
                nc.vector.tensor_tensor(out=val[:, :, 1, :],
                                        in0=val[:, :, 0, :],
                                        in1=ea, op=OP.mult)
                return val

            # ---- layers ----
            for lay in range(L):
                if lay == 0:
                    zsrc = h
                else:
                    ln_partition(zf, h, HID, ngc[:, lay:lay + 1],
                                 nbc[:, lay:lay + 1])
                    zsrc = zf
                zident = ident[:HID, :HID] if lay == 0 else ident_hi[HID:P, :]
                for b in range(NBLK):
                    ptz = ptp.tile([P, HID], F32, name="ptz", tag="tp")
                    zin = (h[:HID, b * P:(b + 1) * P] if lay == 0
                           else zf[:, b * P:(b + 1) * P])
                    nc.tensor.transpose(ptz[:], zin, zident)
                    nc.vector.tensor_copy(zrows[:, b * HID:(b + 1) * HID],
                                          ptz[:])
                nc.vector.tensor_tensor(
                    out=zbb[:],
                    in0=zrows[:].rearrange("p (b f) -> p b f", b=NBLK),
                    in1=benm[:].rearrange("p (o f) -> p o f", o=1)
                        .to_broadcast([P, NBLK, HID]),
                    op=OP.add)
                nc.sync.dma_start(
                    out=zbounce[lay][:].rearrange("(p b) f -> p (b f)", p=P),
                    in_=zbb[:])
                nc.gpsimd.collective_compute(
                    "AllGather", OP.bypass,
                    replica_groups=[list(range(NCORES))],
                    ins=[zbounce[lay][:].opt()],
                    outs=[ztable[lay][:].opt()])
                if dbg and lay == 0:
                    nc.sync.dma_start(out=d_dbg_tab, in_=ztable[lay][:])

                # ---- edge phase ----
                for b in range(NBLK):
                    cbase = choff[b]
                    Kb = K_E[b] + K_O[b]
                    gt = gts[b % 3]
                    nc.gpsimd.dma_gather(
                        out_ap=gt[:, 0:Kb, :],
                        in_ap=ztable[lay][:].rearrange(
                            "(m two) f -> m (two f)", two=2),
                        idxs_ap=idx16[:, cbase * 8:(cbase + Kb) * 8],
                        num_idxs=Kb * P,
                        num_idxs_reg=int(reg[b]),
                        elem_size=2 * HID, single_packet=False,
                        queue_num=b % 4)
                    val_e = edge_values(gt, 0, cbase, K_E[b], KEMAX, 0, "e")
                    val_o = edge_values(gt, K_E[b], cbase + K_E[b], K_O[b],
                                        KOMAX, HID, "o")
                    if True:
                        ps = pmm.tile([P, P], F32, name="ps", tag="mm")
                        chunks = (
                            [(val_e, j, cbase + j) for j in range(K_E[b])] +
                            [(val_o, j, cbase + K_E[b] + j)
                             for j in range(K_O[b])])
                        for ci, (vt, vk, ck) in enumerate(chunks):
                            oh = opool.tile([P, P], BF16, name="oh", tag="oh")
                            nc.vector.tensor_tensor(
                                out=oh[:], in0=edl[:, ck:ck + 1]
                                    .to_broadcast([P, P]),
                                in1=iota_f[:], op=OP.is_equal)
                            nc.tensor.matmul(ps[:], lhsT=oh[:],
                                             rhs=vt[:, vk, :, :],
                                             start=(ci == 0),
                                             stop=(ci == len(chunks) - 1))
                        if dbg and lay == 0 and b == 0:
                            sdump = npool.tile([P, 2 * HID], F32,
                                               name="sdump", tag="sdump")
                            nc.vector.tensor_copy(sdump[:], ps[:])
                            nc.sync.dma_start(out=d_dbg_s, in_=sdump[:])
                        rec = npool.tile([P, HID], F32, name="rec", tag="rec")
                        nc.vector.tensor_scalar(out=rec[:], in0=ps[:, 0:HID],
                                                scalar1=EPS_SM, scalar2=None,
                                                op0=OP.add)
                        nc.vector.reciprocal(rec[:], rec[:])
                        in1 = npool.tile([P, HID], F32, name="in1", tag="in1")
                        nc.vector.tensor_tensor(out=in1[:],
                                                in0=ps[:, HID:2 * HID],
                                                in1=rec[:], op=OP.mult)
                        nc.vector.tensor_tensor(
                            out=in1[:], in0=in1[:],
                            in1=zrows[:, b * HID:(b + 1) * HID], op=OP.add)
                        pti = ptp.tile([HID, P], F32, name="pti", tag="tp")
                        nc.tensor.transpose(pti[:], in1[:], ident[:])
                        nc.vector.tensor_copy(in1T[:HID, b * P:(b + 1) * P],
                                              pti[:])

                # ---- node MLP phase ----
                for c0, w in SLICES:
                    p1 = pmm.tile([P, 512], F32, name="p1", tag="mm")
                    nc.tensor.matmul(p1[:, :w], lhsT=w1[lay][:],
                                     rhs=in1T[:HID, c0:c0 + w],
                                     start=True, stop=True)
                    t1 = mpool.tile([P, 512], F32, name="t1", tag="t1")
                    nc.vector.tensor_scalar(out=t1[:, :w], in0=p1[:, :w],
                                            scalar1=b1c[:, lay:lay + 1],
                                            scalar2=None, op0=OP.add)
                    sq = mpool.tile([P, 512], F32, name="msq", tag="msq")
                    nc.scalar.activation(sq[:, :w], t1[:, :w], ACT.Square)
                    pm = paux.tile([1, 512], F32, name="mpm", tag="aux")
                    pq = paux.tile([1, 512], F32, name="mpq", tag="aux")
                    nc.tensor.matmul(pm[:, :w], lhsT=ones_col[:],
                                     rhs=t1[:, :w], start=True, stop=True)
                    nc.tensor.matmul(pq[:, :w], lhsT=ones_col[:],
                                     rhs=sq[:, :w], start=True, stop=True)
                    stA = spool.tile([1, 512], F32, name="mstA", tag="stA")
                    stB = spool.tile([1, 512], F32, name="mstB", tag="stB")
                    tmp = spool.tile([1, 512], F32, name="msttmp", tag="sttmp")
                    nc.vector.tensor_scalar(out=stA[:, :w], in0=pq[:, :w],
                                            scalar1=1.0 / 128, scalar2=None,
                                            op0=OP.mult)
                    nc.vector.tensor_scalar(out=stB[:, :w], in0=pm[:, :w],
                                            scalar1=1.0 / 128, scalar2=None,
                                            op0=OP.mult)
                    nc.vector.tensor_tensor(out=tmp[:, :w], in0=stB[:, :w],
                                            in1=stB[:, :w], op=OP.mult)
                    nc.vector.tensor_tensor(out=stA[:, :w], in0=stA[:, :w],
                                            in1=tmp[:, :w], op=OP.subtract)
                    nc.scalar.activation(stA[:, :w], stA[:, :w], ACT.Sqrt,
                                         bias=lneps_col[:1, :])
                    nc.vector.reciprocal(stA[:, :w], stA[:, :w])
                    nc.vector.tensor_tensor(out=stB[:, :w], in0=stB[:, :w],
                                            in1=stA[:, :w], op=OP.mult)
                    pA = paux.tile([P, 512], F32, name="mpA", tag="aux")
                    pB = paux.tile([P, 512], F32, name="mpB", tag="aux")
                    nc.tensor.matmul(pA[:, :w], lhsT=ones_row[:],
                                     rhs=stA[:, :w], start=True, stop=True)
                    nc.tensor.matmul(pB[:, :w], lhsT=ones_row[:],
                                     rhs=stB[:, :w], start=True, stop=True)
                    nc.vector.tensor_tensor(out=t1[:, :w], in0=t1[:, :w],
                                            in1=pA[:, :w], op=OP.mult)
                    nc.vector.tensor_tensor(out=t1[:, :w], in0=t1[:, :w],
                                            in1=pB[:, :w], op=OP.subtract)
                    nc.vector.tensor_scalar(out=t1[:, :w], in0=t1[:, :w],
                                            scalar1=g1c[:, lay:lay + 1],
                                            scalar2=be1c[:, lay:lay + 1],
                                            op0=OP.mult, op1=OP.add)
                    nc.scalar.activation(sq[:, :w], t1[:, :w], ACT.Relu)
                    p2 = pmm.tile([HID, 512], F32, name="p2", tag="mm")
                    nc.tensor.matmul(p2[:, :w], lhsT=w2[lay][:], rhs=sq[:, :w],
                                     start=True, stop=True)
                    if lay == 0:
                        nc.vector.tensor_scalar(out=h[:, c0:c0 + w],
                                                in0=p2[:, :w],
                                                scalar1=b2c[:, 0:1],
                                                scalar2=None, op0=OP.add)
                    else:
                        conv = mpool.tile([HID, 512], F32, name="conv",
                                          tag="conv")
                        nc.vector.tensor_scalar(out=conv[:, :w], in0=p2[:, :w],
                                                scalar1=b2c[:, lay:lay + 1],
                                                scalar2=None, op0=OP.add)
                        nc.vector.tensor_tensor(out=h[:, c0:c0 + w],
                                                in0=h[:, c0:c0 + w],
                                                in1=conv[:, :w], op=OP.add)

            # ---- final norm + pooling ----
            ln_partition(zf, h, HID, ngc[:, 0:1], nbc[:, 0:1])
            poolsb = cpool.tile([HID, 2], F32)
            nc.vector.tensor_reduce(out=poolsb[:, 0:1], in_=zf[:HID, :],
                                    axis=mybir.AxisListType.X, op=OP.add)
            nc.vector.tensor_reduce(out=poolsb[:, 1:2], in_=zf[:HID, :],
                                    axis=mybir.AxisListType.X, op=OP.max)
            nc.sync.dma_start(out=d_out, in_=poolsb[:])

    nc.finalize()
    return nc


def make_in_maps(data, idx_c, dl_c, av_c):
    x = data["x"].astype(np.float32)
    xpad = np.zeros((NTOT, 16), np.float32)
    xpad[:N_NODES] = x
    in_maps = []
    for c in range(NCORES):
        xc = xpad[c * NPC:(c + 1) * NPC]
        xp = np.ascontiguousarray(
            xc.reshape(NBLK, P, 16).transpose(1, 0, 2).reshape(NPC, 16))
        in_maps.append({
            "eidx16": idx_c[c], "edl": dl_c[c], "eav": av_c[c], "xp": xp,
            "wnode": np.ascontiguousarray(data["W_node"].astype(np.float32)),
            "bnode": data["b_node"].astype(np.float32)[:, None].copy(),
            "web": np.tile(data["W_edge"].astype(np.float32)[0][None, :],
                           (P, 1)),
            "benm": np.tile(data["b_edge"].astype(np.float32)[None, :],
                            (P, 1)),
            "w1": np.ascontiguousarray(data["W1"].astype(np.float32)),
            "w2": np.ascontiguousarray(data["W2"].astype(np.float32)),
            "b1c": np.ascontiguousarray(data["b1"].astype(np.float32).T),
            "g1c": np.ascontiguousarray(data["ln_g"].astype(np.float32).T),
            "be1c": np.ascontiguousarray(data["ln_b"].astype(np.float32).T),
            "b2c": np.ascontiguousarray(data["b2"].astype(np.float32).T),
            "ngc": np.ascontiguousarray(data["norm_g"].astype(np.float32).T),
            "nbc": np.ascontiguousarray(data["norm_b"].astype(np.float32).T),
            "tcol": np.tile(data["t"].astype(np.float32)[None, :], (P, 1)),
        })
    return in_maps


def kernel(x, edge_attr, edge_index, W_node, b_node, W_edge, b_edge, t,
           W1, b1, ln_g, ln_b, W2, b2, norm_g, norm_b, W_lin, b_lin):
    data = dict(x=np.asarray(x), edge_attr=np.asarray(edge_attr),
                W_node=np.asarray(W_node), b_node=np.asarray(b_node),
                W_edge=np.asarray(W_edge), b_edge=np.asarray(b_edge),
                t=np.asarray(t), W1=np.asarray(W1), b1=np.asarray(b1),
                ln_g=np.asarray(ln_g), ln_b=np.asarray(ln_b),
                W2=np.asarray(W2), b2=np.asarray(b2),
                norm_g=np.asarray(norm_g), norm_b=np.asarray(norm_b))
    K_E, K_O, reg, idx_c, dl_c, av_c = _prep_edges(
        np.asarray(edge_index), np.asarray(edge_attr))
    nc = _build(K_E, K_O, reg)
    in_maps = make_in_maps(data, idx_c, dl_c, av_c)
    res = run_bass_kernel_spmd(nc, in_maps, core_ids=list(range(NCORES)))
    outs = np.stack([r["pool_out"] for r in res.results])  # [8, 64, 2]
    sums = outs[:, :, 0].sum(axis=0)
    maxs = outs[:, :, 1].max(axis=0)
    avg = (sums / float(N_NODES)).reshape(32, 2).mean(axis=1)
    mx = maxs.reshape(32, 2).max(axis=1)
    emb = np.concatenate([avg, mx])[None, :].astype(np.float32)
    out = emb @ np.asarray(W_lin, np.float32) + np.asarray(b_lin, np.float32)
    return out.astype(np.float32)

